# revision 14
# baseline (speedup 1.0000x reference)
"""DecoderRNN Trainium2 kernel: 63-step LSTM + Luong attention + vocab projection.

Strategy (8 NeuronCores, SPMD), fp16 datapath (c-state/PSUM/softmax in f32):
  - Recurrence TP=8 over gate dims: each core owns 128 hidden dims x 4 gates
    (quarter order i,f,o,g so one sigmoid ACT covers i|f|o). Gates accumulate in
    ONE psum tile [P, 4B]; precomputed XgT enters via an identity-matmul.
    Per-step AllGather of the fp16 h-slice; payload [P, 3B] also piggybacks
    dect row-chunks (see below) so no extra collectives are needed.
  - Attention + W_w decoder: processed in t-blocks after the block's h has
    landed, spread across later steps as PE filler inside the AllGather gaps
    (also keeps the PE HAM-warm). W_w output is sharded by hidden chunk per
    core (per-core weight slice); the AllGather piggyback distributes dect so
    every core gets the full [H, R] dect for its vocab slice.
  - Vocab projection V-sharded (4000 cols/core), interleaved into the loop as
    dect rows land; out DMA per (t-group, n-tile) chunk.
  - Host side does layout-only prep; output is np.concatenate over V.
"""

import numpy as np
import ml_dtypes
from contextlib import ExitStack

import concourse.bass as bass
import concourse.bacc as bacc
import concourse.tile as tile
import concourse.mybir as mybir
from concourse import masks
from concourse.bass_utils import run_bass_kernel_spmd

F32 = mybir.dt.float32
F16 = mybir.dt.float16
AF = mybir.ActivationFunctionType
ALU = mybir.AluOpType

B, T, S = 32, 63, 64
V, E, H = 32000, 512, 1024
P = 128
NCORES = 8
R = T * B                      # 2016 rows, r = t*B + b
VL = V // NCORES               # 4000
KH = H // P                    # 8
KE = E // P                    # 4
CH = 4                         # owned gate chunks (i,f,o,g quarters)
NT = 500                       # vocab n-tile width
VN = VL // NT                  # 8
Q_ORDER = [0, 1, 3, 2]         # quarter -> pytorch gate index (i,f,o,g)

# attention blocks (start, end)
BLOCKS = [(0, 16), (16, 32), (32, 44), (44, 56), (56, 63)]
SPREADS = [4, 4, 4, 8, 8]
TGROUPS = [(4 * i, min(4 * i + 4, T)) for i in range(16)]

# ---------------- static schedule ----------------


def build_schedule():
    scores = {}    # step -> list of (blk_idx, b)
    ctxs = {}      # step -> list of (blk_idx, jpair)
    dec = {}       # step -> blk_idx
    ship = {}      # slot -> (d0, nchunks)
    land = {}      # row-chunk d -> landing step
    post_blocks = []
    for bi, (a, bnd) in enumerate(BLOCKS):
        start = bnd + 1
        spread = SPREADS[bi]
        steps_needed = (32 + spread - 1) // spread
        if start + steps_needed + 2 > T:
            post_blocks.append(bi)
            continue
        for j in range(32):
            st = start + j // spread
            scores.setdefault(st, []).append((bi, j))
            if j % 2 == 1:
                ctxs.setdefault(st + 1, []).append((bi, j // 2))
        dstep = start + steps_needed + 1
        dec[dstep] = bi
        d = a
        slot = dstep + 1
        while d < bnd:
            nch = min(2, bnd - d)
            if slot >= T:
                break
            ship[slot] = (d, nch)
            for dd in range(d, d + nch):
                land[dd] = slot + 1
            d += nch
            slot += 1
    tail_rows = [d for d in range(T) if d not in land]
    avail = {}
    for g, (ta, tb) in enumerate(TGROUPS):
        if all(d in land for d in range(ta, tb)):
            avail[g] = max(land[d] for d in range(ta, tb)) + 1
        else:
            avail[g] = None
    vocab = {}
    items = []
    ready_groups = sorted([g for g in avail if avail[g] is not None],
                          key=lambda g: avail[g])
    rounds = []
    cur = []
    for g in ready_groups:
        cur.append(g)
        if len(cur) == 2:
            rounds.append(cur)
            cur = []
    if cur:
        rounds.append(cur)
    for rnd in rounds:
        rstart = max(avail[g] for g in rnd)
        for n in range(VN):
            for i, g in enumerate(rnd):
                items.append((rstart, g, n, i == 0))
    items.sort(key=lambda x: x[0])
    qi = 0
    for t in range(T):
        quota = 2 if t < 40 else 3
        cnt = 0
        while qi < len(items) and cnt < quota and items[qi][0] <= t:
            _, g, n, ld = items[qi]
            vocab.setdefault(t, []).append((g, n, ld))
            qi += 1
            cnt += 1
    tail_vocab_early = []   # runnable before fin AG (rows landed in-loop)
    tail_vocab_late = []
    for (_, g, n, ld) in items[qi:]:
        tail_vocab_early.append((g, n))
    for g in [g for g in avail if avail[g] is None]:
        for n in range(VN):
            tail_vocab_late.append((g, n))
    return (scores, ctxs, dec, ship, tail_rows, vocab,
            tail_vocab_early, tail_vocab_late, post_blocks)


(ATTN_SCHED, CTX_SCHED, DEC_SCHED, SHIP_SCHED, TAIL_ROWS, VOCAB_SCHED,
 TAIL_VOCAB_EARLY, TAIL_VOCAB_LATE, POST_BLOCKS) = build_schedule()
POST_BLOCKS = sorted(set(POST_BLOCKS))
STAGEA_STEPS = {2: 1, 8: 2, 12: 3}   # step -> stage-A window (window 0 pre-loop)
AW = [(0, 512), (512, 1024), (1024, 1536), (1536, 2016)]


def build_graph():
    nc = bacc.Bacc("TRN2", target_bir_lowering=False, debug=False,
                   num_devices=NCORES)

    def inp(name, shape, dtype):
        return nc.dram_tensor(name, list(shape), dtype, kind="ExternalInput").ap()

    x_embT = inp("x_embT", [E, R], F16)
    wih_s = inp("wih_s", [E, CH * P], F16)
    whh_s = inp("whh_s", [H, CH * P], F16)
    bias_s = inp("bias_s", [P, CH], F32)
    h0T = inp("h0T", [H, B], F16)
    c0T_s = inp("c0T_s", [P, B], F32)
    encT_r = inp("encT_r", [P, B * KH * S], F16)   # [p, b, k, s]
    enc_r = inp("enc_r", [B, S, H], F16)
    ww_s = inp("ww_s", [2 * H, P], F16)            # W_w.T cols for own mo chunk
    bw_s = inp("bw_s", [P, 1], F32)
    wout_s = inp("wout_s", [H, VL], F16)
    bout_s = inp("bout_s", [1, VL], F16)
    out_s = nc.dram_tensor("out_s", [B, T, VL], F32, kind="ExternalOutput").ap()

    with tile.TileContext(nc) as tc, ExitStack() as ctx:
        pool1 = ctx.enter_context(tc.tile_pool(name="pool1", bufs=1))
        stream = ctx.enter_context(tc.tile_pool(name="stream", bufs=3))
        work = ctx.enter_context(tc.tile_pool(name="work", bufs=2))
        state = ctx.enter_context(tc.tile_pool(name="state", bufs=2))
        psp = ctx.enter_context(tc.tile_pool(name="psp", bufs=1, space="PSUM"))
        dram = ctx.enter_context(tc.tile_pool(name="dram", bufs=1, space="DRAM"))

        # ---------------- resident tiles ----------------
        hall = pool1.tile([P, KH, R], F16, name="hall")
        hall4 = hall.rearrange("p k (t b) -> p k t b", b=B)
        dectT = pool1.tile([P, KH, R], F16, name="dectT")
        dect_own = pool1.tile([P, R], F16, name="dect_own")
        whh = pool1.tile([P, KH, CH * P], F16, name="whh")
        nc.sync.dma_start(whh[:], whh_s.rearrange("(k p) c -> p k c", p=P))
        wih = pool1.tile([P, KE, CH * P], F16, name="wih")
        nc.sync.dma_start(wih[:], wih_s.rearrange("(k p) c -> p k c", p=P))
        bias_t = pool1.tile([P, CH], F32, name="bias_t")
        nc.sync.dma_start(bias_t[:], bias_s[:])
        encT_sb = pool1.tile([P, B, KH, S], F16, name="encT_sb")
        nc.sync.dma_start(encT_sb[:],
                          encT_r.rearrange("p (b k s) -> p b k s", b=B, k=KH))
        ww_sb = pool1.tile([P, 2 * KH, P], F16, name="ww_sb")
        nc.sync.dma_start(ww_sb[:], ww_s.rearrange("(j p) m -> p j m", p=P))
        bw_t = pool1.tile([P, 1], F32, name="bw_t")
        nc.sync.dma_start(bw_t[:], bw_s[:])
        bout_t = pool1.tile([1, VL], F16, name="bout_t")
        nc.sync.dma_start(bout_t[:], bout_s[:])
        ones_t = pool1.tile([1, P], F16, name="ones_t")
        nc.gpsimd.memset(ones_t[:], 1.0)
        h0_t = pool1.tile([P, KH, B], F16, name="h0_t")
        nc.sync.dma_start(h0_t[:], h0T.rearrange("(k p) b -> p k b", p=P))
        ident = pool1.tile([P, P], F16, name="ident")
        masks.make_identity(nc, ident[:])
        c0_sb = pool1.tile([P, B], F32, name="c0_sb")
        nc.sync.dma_start(c0_sb[:], c0T_s[:])

        xg_dram = dram.tile([CH, P, R], F16, name="xg_dram")
        cc_in = [dram.tile([P, 3 * B], F16, name=f"cc_in{i}") for i in range(T)]
        cc_out = [dram.tile([NCORES * P, 3 * B], F16, name=f"cc_out{i}",
                            addr_space="Shared") for i in range(T)]
        NTAIL = len(TAIL_ROWS)
        fin_in = dram.tile([P, NTAIL * B], F16, name="fin_in")
        fin_out = dram.tile([NCORES * P, NTAIL * B], F16, name="fin_out",
                            addr_space="Shared")

        # ---------------- helpers ----------------
        def stage_a(w):
            a, bnd = AW[w]
            nw = bnd - a
            xt = stream.tile([P, KE, 512], F16, name="xa", tag="xa", bufs=2)
            nc.gpsimd.dma_start(xt[:, :, :nw],
                                x_embT.rearrange("(k p) r -> p k r", p=P)[:, :, a:bnd])
            for c in range(CH):
                ps = psp.tile([P, 512], F32, name="ps_a", tag="mm")
                for k in range(KE):
                    nc.tensor.matmul(ps[:, :nw], lhsT=wih[:, k, c * P:(c + 1) * P],
                                     rhs=xt[:, k, :nw],
                                     start=(k == 0), stop=(k == KE - 1))
                xga = work.tile([P, 512], F16, name="xga", tag="xga", bufs=2)
                nc.scalar.activation(xga[:, :nw], ps[:, :nw], AF.Identity,
                                     bias=bias_t[:, c:c + 1])
                nc.gpsimd.dma_start(xg_dram[c, :, a:bnd], xga[:, :nw])

        def xg_prefetch(t):
            xg = stream.tile([P, CH, B], F16, name="xg", tag="xg", bufs=4)
            nc.gpsimd.dma_start(
                xg[:], xg_dram[:, :, t * B:(t + 1) * B].rearrange("c p b -> p c b"))
            return xg

        ec_tiles = {}
        pn2_tiles = {}

        def attn_scores(bi, j):
            blk_a, blk_b = BLOCKS[bi]
            w = blk_b - blk_a
            if j % 2 == 0 and (bi, j // 2) not in ec_tiles:
                b0 = j
                ec = stream.tile([2 * S, H], F16, name="ec", tag="ec", bufs=4)
                nc.gpsimd.dma_start(ec[0:S, :], enc_r[b0, :, :])
                nc.gpsimd.dma_start(ec[S:2 * S, :], enc_r[b0 + 1, :, :])
                ec_tiles[(bi, j // 2)] = ec
            ps_sc = psp.tile([P, S], F32, name="ps_sc", tag="mm")
            for k in range(KH):
                nc.tensor.matmul(ps_sc[:w, :],
                                 lhsT=hall4[:, k, blk_a:blk_b, j],
                                 rhs=encT_sb[:, j, k, :],
                                 start=(k == 0), stop=(k == KH - 1))
            probs = work.tile([P, S], F32, name="probs", tag="probs")
            ssum = work.tile([P, 1], F32, name="ssum", tag="ssum")
            nc.scalar.activation(probs[:w, :], ps_sc[:w, :], AF.Exp,
                                 accum_out=ssum[:w])
            lgd = work.tile([P, 1], F32, name="lgd", tag="lgd")
            nc.scalar.activation(lgd[:w], ssum[:w], AF.Ln)
            rec = work.tile([P, 1], F32, name="rec", tag="rec")
            nc.scalar.activation(rec[:w], lgd[:w], AF.Exp, scale=-1.0)
            pn2 = pn2_tiles.get((bi, j // 2))
            if pn2 is None:
                pn2 = work.tile([P, 2, S], F16, name="pn2", tag="pn2", bufs=6)
                pn2_tiles[(bi, j // 2)] = pn2
            nc.scalar.mul(pn2[:w, j % 2, :], probs[:w, :], rec[:w])

        def attn_ctx(bi, jp):
            blk_a, blk_b = BLOCKS[bi]
            w = blk_b - blk_a
            ec = ec_tiles.pop((bi, jp))
            pn2 = pn2_tiles.pop((bi, jp))
            ps_at = psp.tile([P, 16], F16, name="ps_at", tag="at")
            nc.tensor.transpose(
                ps_at[:, :w],
                pn2.rearrange("p a s -> p (a s)")[:w, :],
                ident[:w, :w])
            attnT = work.tile([P, 16], F16, name="attnT", tag="attnT", bufs=2)
            nc.vector.tensor_copy(attnT[:, :w], ps_at[:, :w])
            for jj in range(2):
                bb = jp * 2 + jj
                ps_cx = psp.tile([P, KH, 16], F32, name="ps_cx", tag="cx")
                for k in range(KH):
                    nc.tensor.matmul(ps_cx[:, k, :w],
                                     lhsT=ec[jj * S:(jj + 1) * S,
                                             k * P:(k + 1) * P],
                                     rhs=attnT[jj * S:(jj + 1) * S, :w],
                                     start=True, stop=True)
                cxb = ctx_blk[bi % 2]
                cxr = cxb.rearrange("p k (t b) -> p k t b", b=B)
                nc.vector.tensor_copy(cxr[:, :, :w, bb], ps_cx[:, :, :w])

        def dec_blk(bi):
            blk_a, blk_b = BLOCKS[bi]
            w = blk_b - blk_a
            cxb = ctx_blk[bi % 2]
            ps_d = psp.tile([P, 512], F32, name="ps_d", tag="dec")
            for j in range(2 * KH):
                rhs = (hall[:, j, blk_a * B:blk_b * B] if j < KH
                       else cxb[:, j - KH, :w * B])
                nc.tensor.matmul(ps_d[:, :w * B], lhsT=ww_sb[:, j, :], rhs=rhs,
                                 start=(j == 0), stop=(j == 2 * KH - 1))
            nc.scalar.activation(dect_own[:, blk_a * B:blk_b * B], ps_d[:, :w * B],
                                 AF.Tanh, bias=bw_t[:, 0:1])

        wo_tiles = {}

        def vocab_chunk(g, n, load):
            ta, tb = TGROUPS[g]
            mw = (tb - ta) * B
            wo = wo_tiles.get(n % 4) if not load else None
            if load or wo is None:
                wo = stream.tile([P, KH, NT], F16, name="wo", tag=f"wo{n % 4}",
                                 bufs=1)
                nc.gpsimd.dma_start(
                    wo[:], wout_s[:, n * NT:(n + 1) * NT]
                    .rearrange("(k p) v -> p k v", p=P))
                wo_tiles[n % 4] = wo
            ps_v = psp.tile([P, NT], F32, name="ps_v", tag="pv", bufs=3)
            for k in range(KH):
                nc.tensor.matmul(ps_v[:mw, :], lhsT=dectT[:, k, ta * B:tb * B],
                                 rhs=wo[:, k, :], start=(k == 0), stop=False)
            nc.tensor.matmul(ps_v[:mw, :], lhsT=ones_t[0:1, :mw],
                             rhs=bout_t[0:1, n * NT:(n + 1) * NT],
                             start=False, stop=True)
            o_sb = work.tile([P, NT], F32, name="o_sb", tag="o_sb", bufs=3)
            nc.vector.tensor_copy(o_sb[:mw, :], ps_v[:mw, :])
            nc.gpsimd.dma_start(
                out_s[:, ta:tb, n * NT:(n + 1) * NT].transpose([1, 0, 2]),
                o_sb[:mw, :])

        # ---------------- pre-loop ----------------
        ctx_blk = [pool1.tile([P, KH, 16 * B], F16, name=f"cxb{i}")
                   for i in range(2)]
        stage_a(0)
        xg_q = {0: xg_prefetch(0), 1: xg_prefetch(1)}

        # ---------------- main loop ----------------
        c_prev = c0_sb
        for t in range(T):
            # gates: psum [P, 4B]; identity-matmul folds Xg in
            psg = psp.tile([P, CH * B], F32, name="psg", tag="psg", bufs=1)
            xg = xg_q.pop(t)
            nc.tensor.matmul(psg[:], lhsT=ident[:],
                             rhs=xg[:].rearrange("p c b -> p (c b)"),
                             start=True, stop=False, skip_group_check=True)
            for qq in range(CH):
                for k in range(KH):
                    rhs = (h0_t[:, k, :] if t == 0 else
                           hall4[:, k, t - 1, :])
                    nc.tensor.matmul(psg[:, qq * B:(qq + 1) * B],
                                     lhsT=whh[:, k, qq * P:(qq + 1) * P],
                                     rhs=rhs, start=False,
                                     stop=(qq == CH - 1 and k == KH - 1),
                                     skip_group_check=True)
            sfo = work.tile([P, 3 * B], F32, name="sfo", tag="sfo")
            nc.scalar.activation(sfo[:], psg[:, 0:3 * B], AF.Sigmoid)
            tg = work.tile([P, B], F32, name="tg", tag="tg")
            nc.scalar.activation(tg[:], psg[:, 3 * B:4 * B], AF.Tanh)
            t1 = work.tile([P, B], F32, name="t1", tag="t1")
            nc.gpsimd.tensor_mul(t1[:], sfo[:, B:2 * B], c_prev[:])
            t2 = work.tile([P, B], F32, name="t2", tag="t2")
            nc.gpsimd.tensor_mul(t2[:], sfo[:, 0:B], tg[:])
            c_new = state.tile([P, B], F32, name="c_new", tag="c_new")
            nc.gpsimd.tensor_add(c_new[:], t1[:], t2[:])
            c_prev = c_new
            tc_t = work.tile([P, B], F32, name="tc_t", tag="tc_t")
            nc.scalar.activation(tc_t[:], c_new[:], AF.Tanh)
            h16 = work.tile([P, B], F16, name="h16", tag="h16")
            nc.gpsimd.tensor_mul(h16[:], sfo[:, 2 * B:3 * B], tc_t[:])
            nc.gpsimd.dma_start(cc_in[t][:, 0:B], h16[:])
            nc.gpsimd.collective_compute(
                "AllGather", ALU.bypass,
                replica_groups=[list(range(NCORES))],
                ins=[cc_in[t].opt()], outs=[cc_out[t].opt()])
            nc.sync.dma_start(
                hall4[:, :, t, :],
                cc_out[t][:, 0:B].rearrange("(k p) b -> p k b", p=P))
            if t in SHIP_SCHED:
                d0, nch = SHIP_SCHED[t]
                nc.sync.dma_start(
                    dectT[:, :, d0 * B:(d0 + nch) * B],
                    cc_out[t][:, B:(1 + nch) * B]
                    .rearrange("(k p) b -> p k b", p=P))

            # ---- filler ----
            if t + 2 < T:
                xg_q[t + 2] = xg_prefetch(t + 2)
            if t in STAGEA_STEPS:
                stage_a(STAGEA_STEPS[t])
            for (bi, j) in ATTN_SCHED.get(t, []):
                attn_scores(bi, j)
            for (bi, jp) in CTX_SCHED.get(t, []):
                attn_ctx(bi, jp)
            if t in DEC_SCHED:
                dec_blk(DEC_SCHED[t])
            if t + 1 in SHIP_SCHED:
                d0, nch = SHIP_SCHED[t + 1]
                nc.gpsimd.dma_start(cc_in[t + 1][:, B:(1 + nch) * B],
                                    dect_own[:, d0 * B:(d0 + nch) * B])
            for (g, n, ld) in VOCAB_SCHED.get(t, []):
                vocab_chunk(g, n, ld)

        # ---------------- tail ----------------
        # interleave post-block attention with vocab whose rows landed in-loop
        early = list(TAIL_VOCAB_EARLY)
        ei = 0
        for bi in POST_BLOCKS:
            for j in range(32):
                attn_scores(bi, j)
                if j % 2 == 1:
                    attn_ctx(bi, j // 2)
                if ei < len(early) and j % 2 == 1:
                    g, n = early[ei]
                    vocab_chunk(g, n, True)
                    ei += 1
            dec_blk(bi)
        for (g, n) in early[ei:]:
            vocab_chunk(g, n, True)
        assert TAIL_ROWS == list(range(TAIL_ROWS[0], TAIL_ROWS[-1] + 1))
        nc.sync.dma_start(
            fin_in[:], dect_own[:, TAIL_ROWS[0] * B:(TAIL_ROWS[-1] + 1) * B])
        nc.gpsimd.collective_compute(
            "AllGather", ALU.bypass, replica_groups=[list(range(NCORES))],
            ins=[fin_in.opt()], outs=[fin_out.opt()])
        nc.sync.dma_start(
            dectT[:, :, TAIL_ROWS[0] * B:(TAIL_ROWS[-1] + 1) * B],
            fin_out[:].rearrange("(k p) b -> p k b", p=P))
        tail_by_n = {}
        for (g, n) in TAIL_VOCAB_LATE:
            tail_by_n.setdefault(n, []).append(g)
        for n, gs in sorted(tail_by_n.items()):
            for i, g in enumerate(sorted(set(gs))):
                vocab_chunk(g, n, i == 0)
    nc.compile()
    return nc


_CACHE = {}


def _get_graph():
    if "nc" not in _CACHE:
        _CACHE["nc"] = build_graph()
    return _CACHE["nc"]


def _prep(tgt_input, hidden_state, cell_state, encoder_outputs,
          embedding, W_ih, W_hh, b_ih, b_hh, W_w, b_w, W_out, b_out):
    f32 = np.float32
    f16 = np.float16
    idx = np.asarray(tgt_input)[:, :-1].astype(np.int64)
    emb = np.asarray(embedding, f32)[idx]                    # [B, T, E]
    x_embT = np.ascontiguousarray(
        emb.transpose(2, 1, 0).reshape(E, R)).astype(f16)
    w_ihT = np.asarray(W_ih, f32).T                          # [E, G]
    w_hhT = np.asarray(W_hh, f32).T                          # [H, G]
    bias = (np.asarray(b_ih, f32) + np.asarray(b_hh, f32))
    h0T = np.ascontiguousarray(np.asarray(hidden_state, f32)[0].T).astype(f16)
    c0T = np.ascontiguousarray(np.asarray(cell_state, f32)[0].T)   # [H, B]
    enc = np.asarray(encoder_outputs, f32)                   # [B, S, H]
    enc_r = enc.astype(f16)
    encT_r = np.ascontiguousarray(
        enc.transpose(2, 1, 0)                               # [H, S, B]
        .reshape(KH, P, S, B).transpose(1, 3, 0, 2)          # [P, B, KH, S]
        .reshape(P, B * KH * S)).astype(f16)
    w_wT = np.ascontiguousarray(np.asarray(W_w, f32).T)      # [2H, H]
    b_w_a = np.asarray(b_w, f32)
    w_outT = np.asarray(W_out, f32).T                        # [H, V]
    b_out_a = np.asarray(b_out, f32)

    in_maps = []
    for m in range(NCORES):
        cols = np.concatenate([np.arange(Q_ORDER[q] * H + m * P,
                                         Q_ORDER[q] * H + m * P + P)
                               for q in range(4)])
        in_maps.append({
            "x_embT": x_embT,
            "wih_s": np.ascontiguousarray(w_ihT[:, cols]).astype(f16),
            "whh_s": np.ascontiguousarray(w_hhT[:, cols]).astype(f16),
            "bias_s": np.ascontiguousarray(bias[cols].reshape(CH, P).T),
            "h0T": h0T,
            "c0T_s": np.ascontiguousarray(c0T[m * P:(m + 1) * P, :]),
            "encT_r": encT_r,
            "enc_r": enc_r,
            "ww_s": np.ascontiguousarray(w_wT[:, m * P:(m + 1) * P]).astype(f16),
            "bw_s": np.ascontiguousarray(b_w_a[m * P:(m + 1) * P]).reshape(P, 1),
            "wout_s": np.ascontiguousarray(
                w_outT[:, m * VL:(m + 1) * VL]).astype(f16),
            "bout_s": np.ascontiguousarray(
                b_out_a[m * VL:(m + 1) * VL]).reshape(1, VL).astype(f16),
        })
    return in_maps


def kernel(**inputs) -> np.ndarray:
    nc = _get_graph()
    in_maps = _prep(**inputs)
    res = run_bass_kernel_spmd(nc, in_maps, list(range(NCORES)))
    outs = [res.results[m]["out_s"] for m in range(NCORES)]
    return np.concatenate(outs, axis=2)


# revision 15
# speedup vs baseline: 1.0907x; 1.0907x over previous
"""DecoderRNN Trainium2 kernel: 63-step LSTM + Luong attention + vocab projection.

Strategy (8 NeuronCores, SPMD), fp16 datapath (c-state/PSUM/softmax in f32):
  - Recurrence TP=8 over gate dims: each core owns 128 hidden dims x 4 gates
    (quarter order i,f,o,g so one sigmoid ACT covers i|f|o). Gates accumulate in
    ONE psum tile [P, 4B]; precomputed XgT enters via an identity-matmul.
    Per-step AllGather of the fp16 h-slice; payload [P, 3B] also piggybacks
    dect row-chunks (see below) so no extra collectives are needed.
  - Attention + W_w decoder: processed in t-blocks after the block's h has
    landed, spread across later steps as PE filler inside the AllGather gaps
    (also keeps the PE HAM-warm). W_w output is sharded by hidden chunk per
    core (per-core weight slice); the AllGather piggyback distributes dect so
    every core gets the full [H, R] dect for its vocab slice.
  - Vocab projection V-sharded (4000 cols/core), interleaved into the loop as
    dect rows land; out DMA per (t-group, n-tile) chunk.
  - Host side does layout-only prep; output is np.concatenate over V.
"""

import numpy as np
import ml_dtypes
from contextlib import ExitStack

import concourse.bass as bass
import concourse.bacc as bacc
import concourse.tile as tile
import concourse.mybir as mybir
from concourse import masks
from concourse.bass_utils import run_bass_kernel_spmd

F32 = mybir.dt.float32
F16 = mybir.dt.float16
AF = mybir.ActivationFunctionType
ALU = mybir.AluOpType

B, T, S = 32, 63, 64
V, E, H = 32000, 512, 1024
P = 128
NCORES = 8
R = T * B                      # 2016 rows, r = t*B + b
VL = V // NCORES               # 4000
KH = H // P                    # 8
KE = E // P                    # 4
CH = 4                         # owned gate chunks (i,f,o,g quarters)
NT = 500                       # vocab n-tile width
VN = VL // NT                  # 8
Q_ORDER = [0, 1, 3, 2]         # quarter -> pytorch gate index (i,f,o,g)

# attention blocks (start, end)
BLOCKS = [(0, 16), (16, 32), (32, 44), (44, 56), (56, 63)]
SPREADS = [4, 4, 4, 8, 8]
TGROUPS = [(4 * i, min(4 * i + 4, T)) for i in range(16)]

# ---------------- static schedule ----------------


def build_schedule():
    scores = {}    # step -> list of (blk_idx, b)
    ctxs = {}      # step -> list of (blk_idx, jpair)
    dec = {}       # step -> blk_idx
    ship = {}      # slot -> (d0, nchunks)
    land = {}      # row-chunk d -> landing step
    post_blocks = []
    for bi, (a, bnd) in enumerate(BLOCKS):
        start = bnd + 1
        spread = SPREADS[bi]
        steps_needed = (32 + spread - 1) // spread
        if start + steps_needed + 2 > T:
            post_blocks.append(bi)
            continue
        for j in range(32):
            st = start + j // spread
            scores.setdefault(st, []).append((bi, j))
            if j % 2 == 1:
                ctxs.setdefault(st + 1, []).append((bi, j // 2))
        dstep = start + steps_needed + 1
        dec[dstep] = bi
        d = a
        slot = dstep + 1
        while d < bnd:
            nch = min(2, bnd - d)
            if slot >= T:
                break
            ship[slot] = (d, nch)
            for dd in range(d, d + nch):
                land[dd] = slot + 1
            d += nch
            slot += 1
    tail_rows = [d for d in range(T) if d not in land]
    avail = {}
    for g, (ta, tb) in enumerate(TGROUPS):
        if all(d in land for d in range(ta, tb)):
            avail[g] = max(land[d] for d in range(ta, tb)) + 1
        else:
            avail[g] = None
    vocab = {}
    items = []
    ready_groups = sorted([g for g in avail if avail[g] is not None],
                          key=lambda g: avail[g])
    rounds = []
    cur = []
    for g in ready_groups:
        cur.append(g)
        if len(cur) == 2:
            rounds.append(cur)
            cur = []
    if cur:
        rounds.append(cur)
    for rnd in rounds:
        rstart = max(avail[g] for g in rnd)
        for n in range(VN):
            for i, g in enumerate(rnd):
                items.append((rstart, g, n, i == 0))
    items.sort(key=lambda x: x[0])
    qi = 0
    for t in range(T):
        quota = 2 if t < 40 else 3
        cnt = 0
        while qi < len(items) and cnt < quota and items[qi][0] <= t:
            _, g, n, ld = items[qi]
            vocab.setdefault(t, []).append((g, n, ld))
            qi += 1
            cnt += 1
    tail_vocab_early = []   # runnable before fin AG (rows landed in-loop)
    tail_vocab_late = []
    for (_, g, n, ld) in items[qi:]:
        tail_vocab_early.append((g, n))
    for g in [g for g in avail if avail[g] is None]:
        for n in range(VN):
            tail_vocab_late.append((g, n))
    return (scores, ctxs, dec, ship, tail_rows, vocab,
            tail_vocab_early, tail_vocab_late, post_blocks)


(ATTN_SCHED, CTX_SCHED, DEC_SCHED, SHIP_SCHED, TAIL_ROWS, VOCAB_SCHED,
 TAIL_VOCAB_EARLY, TAIL_VOCAB_LATE, POST_BLOCKS) = build_schedule()
POST_BLOCKS = sorted(set(POST_BLOCKS))
STAGEA_STEPS = {2: 1, 8: 2, 12: 3}   # step -> stage-A window (window 0 pre-loop)
AW = [(0, 512), (512, 1024), (1024, 1536), (1536, 2016)]


def build_graph():
    nc = bacc.Bacc("TRN2", target_bir_lowering=False, debug=False,
                   num_devices=NCORES)

    def inp(name, shape, dtype):
        return nc.dram_tensor(name, list(shape), dtype, kind="ExternalInput").ap()

    x_embT = inp("x_embT", [E, R], F16)
    wih_s = inp("wih_s", [E, CH * P], F16)
    whh_s = inp("whh_s", [H, CH * P], F16)
    bias_s = inp("bias_s", [P, CH], F32)
    h0T = inp("h0T", [H, B], F16)
    c0T_s = inp("c0T_s", [P, B], F32)
    encT_r = inp("encT_r", [P, B * KH * S], F16)   # [p, b, k, s]
    enc_r = inp("enc_r", [B, S, H], F16)
    ww_s = inp("ww_s", [2 * H, P], F16)            # W_w.T cols for own mo chunk
    bw_s = inp("bw_s", [P, 1], F32)
    wout_s = inp("wout_s", [H, VL], F16)
    bout_s = inp("bout_s", [1, VL], F16)
    out_s = nc.dram_tensor("out_s", [B, T, VL], F32, kind="ExternalOutput").ap()

    with tile.TileContext(nc) as tc, ExitStack() as ctx:
        pool1 = ctx.enter_context(tc.tile_pool(name="pool1", bufs=1))
        stream = ctx.enter_context(tc.tile_pool(name="stream", bufs=3))
        work = ctx.enter_context(tc.tile_pool(name="work", bufs=2))
        state = ctx.enter_context(tc.tile_pool(name="state", bufs=2))
        psp = ctx.enter_context(tc.tile_pool(name="psp", bufs=1, space="PSUM"))
        dram = ctx.enter_context(tc.tile_pool(name="dram", bufs=1, space="DRAM"))

        # ---------------- resident tiles ----------------
        hall = pool1.tile([P, KH, R], F16, name="hall")
        hall4 = hall.rearrange("p k (t b) -> p k t b", b=B)
        dectT = pool1.tile([P, KH, R], F16, name="dectT")
        dect_own = pool1.tile([P, R], F16, name="dect_own")
        whh = pool1.tile([P, KH, CH * P], F16, name="whh")
        nc.sync.dma_start(whh[:], whh_s.rearrange("(k p) c -> p k c", p=P))
        wih = pool1.tile([P, KE, CH * P], F16, name="wih")
        nc.sync.dma_start(wih[:], wih_s.rearrange("(k p) c -> p k c", p=P))
        bias_t = pool1.tile([P, CH], F32, name="bias_t")
        nc.sync.dma_start(bias_t[:], bias_s[:])
        encT_sb = pool1.tile([P, B, KH, S], F16, name="encT_sb")
        nc.sync.dma_start(encT_sb[:],
                          encT_r.rearrange("p (b k s) -> p b k s", b=B, k=KH))
        ww_sb = pool1.tile([P, 2 * KH, P], F16, name="ww_sb")
        nc.sync.dma_start(ww_sb[:], ww_s.rearrange("(j p) m -> p j m", p=P))
        bw_t = pool1.tile([P, 1], F32, name="bw_t")
        nc.sync.dma_start(bw_t[:], bw_s[:])
        bout_t = pool1.tile([1, VL], F16, name="bout_t")
        nc.sync.dma_start(bout_t[:], bout_s[:])
        ones_t = pool1.tile([1, P], F16, name="ones_t")
        nc.gpsimd.memset(ones_t[:], 1.0)
        h0_t = pool1.tile([P, KH, B], F16, name="h0_t")
        nc.sync.dma_start(h0_t[:], h0T.rearrange("(k p) b -> p k b", p=P))
        ident = pool1.tile([P, P], F16, name="ident")
        masks.make_identity(nc, ident[:])
        c0_sb = pool1.tile([P, B], F32, name="c0_sb")
        nc.sync.dma_start(c0_sb[:], c0T_s[:])

        xg_dram = dram.tile([CH, P, R], F16, name="xg_dram")
        cc_in = [dram.tile([P, 3 * B], F16, name=f"cc_in{i}") for i in range(T)]
        cc_out = [dram.tile([NCORES * P, 3 * B], F16, name=f"cc_out{i}",
                            addr_space="Shared") for i in range(T)]
        NTAIL = len(TAIL_ROWS)
        fin_in = dram.tile([P, NTAIL * B], F16, name="fin_in")
        fin_out = dram.tile([NCORES * P, NTAIL * B], F16, name="fin_out",
                            addr_space="Shared")

        # ---------------- helpers ----------------
        def stage_a(w):
            a, bnd = AW[w]
            nw = bnd - a
            xt = stream.tile([P, KE, 512], F16, name="xa", tag="xa", bufs=2)
            nc.gpsimd.dma_start(xt[:, :, :nw],
                                x_embT.rearrange("(k p) r -> p k r", p=P)[:, :, a:bnd])
            for c in range(CH):
                ps = psp.tile([P, 512], F32, name="ps_a", tag="mm")
                for k in range(KE):
                    nc.tensor.matmul(ps[:, :nw], lhsT=wih[:, k, c * P:(c + 1) * P],
                                     rhs=xt[:, k, :nw],
                                     start=(k == 0), stop=(k == KE - 1))
                xga = work.tile([P, 512], F16, name="xga", tag="xga", bufs=2)
                nc.scalar.activation(xga[:, :nw], ps[:, :nw], AF.Identity,
                                     bias=bias_t[:, c:c + 1])
                nc.gpsimd.dma_start(xg_dram[c, :, a:bnd], xga[:, :nw])

        def xg_prefetch(t):
            xg = stream.tile([P, CH, B], F16, name="xg", tag="xg", bufs=4)
            nc.gpsimd.dma_start(
                xg[:], xg_dram[:, :, t * B:(t + 1) * B].rearrange("c p b -> p c b"))
            return xg

        ec_tiles = {}
        pn2_tiles = {}

        def attn_scores(bi, j):
            blk_a, blk_b = BLOCKS[bi]
            w = blk_b - blk_a
            if j % 2 == 0 and (bi, j // 2) not in ec_tiles:
                b0 = j
                ec = stream.tile([2 * S, H], F16, name="ec", tag="ec", bufs=4)
                nc.gpsimd.dma_start(ec[0:S, :], enc_r[b0, :, :])
                nc.gpsimd.dma_start(ec[S:2 * S, :], enc_r[b0 + 1, :, :])
                ec_tiles[(bi, j // 2)] = ec
            ps_sc = psp.tile([P, S], F32, name="ps_sc", tag="mm")
            for k in range(KH):
                nc.tensor.matmul(ps_sc[:w, :],
                                 lhsT=hall4[:, k, blk_a:blk_b, j],
                                 rhs=encT_sb[:, j, k, :],
                                 start=(k == 0), stop=(k == KH - 1))
            probs = work.tile([P, S], F32, name="probs", tag="probs")
            ssum = work.tile([P, 1], F32, name="ssum", tag="ssum")
            nc.scalar.activation(probs[:w, :], ps_sc[:w, :], AF.Exp,
                                 accum_out=ssum[:w])
            lgd = work.tile([P, 1], F32, name="lgd", tag="lgd")
            nc.scalar.activation(lgd[:w], ssum[:w], AF.Ln)
            rec = work.tile([P, 1], F32, name="rec", tag="rec")
            nc.scalar.activation(rec[:w], lgd[:w], AF.Exp, scale=-1.0)
            pn2 = pn2_tiles.get((bi, j // 2))
            if pn2 is None:
                pn2 = work.tile([P, 2, S], F16, name="pn2", tag="pn2", bufs=6)
                pn2_tiles[(bi, j // 2)] = pn2
            nc.scalar.mul(pn2[:w, j % 2, :], probs[:w, :], rec[:w])

        def attn_ctx(bi, jp):
            blk_a, blk_b = BLOCKS[bi]
            w = blk_b - blk_a
            ec = ec_tiles.pop((bi, jp))
            pn2 = pn2_tiles.pop((bi, jp))
            ps_at = psp.tile([P, 16], F16, name="ps_at", tag="at")
            nc.tensor.transpose(
                ps_at[:, :w],
                pn2.rearrange("p a s -> p (a s)")[:w, :],
                ident[:w, :w])
            attnT = work.tile([P, 16], F16, name="attnT", tag="attnT", bufs=2)
            nc.vector.tensor_copy(attnT[:, :w], ps_at[:, :w])
            for jj in range(2):
                bb = jp * 2 + jj
                ps_cx = psp.tile([P, KH, 16], F32, name="ps_cx", tag="cx")
                for k in range(KH):
                    nc.tensor.matmul(ps_cx[:, k, :w],
                                     lhsT=ec[jj * S:(jj + 1) * S,
                                             k * P:(k + 1) * P],
                                     rhs=attnT[jj * S:(jj + 1) * S, :w],
                                     start=True, stop=True)
                cxb = ctx_blk[bi % 2]
                cxr = cxb.rearrange("p k (t b) -> p k t b", b=B)
                nc.vector.tensor_copy(cxr[:, :, :w, bb], ps_cx[:, :, :w])

        def dec_blk(bi):
            blk_a, blk_b = BLOCKS[bi]
            w = blk_b - blk_a
            cxb = ctx_blk[bi % 2]
            ps_d = psp.tile([P, 512], F32, name="ps_d", tag="dec")
            for j in range(2 * KH):
                rhs = (hall[:, j, blk_a * B:blk_b * B] if j < KH
                       else cxb[:, j - KH, :w * B])
                nc.tensor.matmul(ps_d[:, :w * B], lhsT=ww_sb[:, j, :], rhs=rhs,
                                 start=(j == 0), stop=(j == 2 * KH - 1))
            nc.scalar.activation(dect_own[:, blk_a * B:blk_b * B], ps_d[:, :w * B],
                                 AF.Tanh, bias=bw_t[:, 0:1])

        wo_tiles = {}

        def vocab_chunk(g, n, load):
            ta, tb = TGROUPS[g]
            mw = (tb - ta) * B
            wo = wo_tiles.get(n % 4) if not load else None
            if load or wo is None:
                wo = stream.tile([P, KH, NT], F16, name="wo", tag=f"wo{n % 4}",
                                 bufs=1)
                nc.gpsimd.dma_start(
                    wo[:], wout_s[:, n * NT:(n + 1) * NT]
                    .rearrange("(k p) v -> p k v", p=P))
                wo_tiles[n % 4] = wo
            ps_v = psp.tile([P, NT], F32, name="ps_v", tag="pv", bufs=3)
            for k in range(KH):
                nc.tensor.matmul(ps_v[:mw, :], lhsT=dectT[:, k, ta * B:tb * B],
                                 rhs=wo[:, k, :], start=(k == 0), stop=False)
            nc.tensor.matmul(ps_v[:mw, :], lhsT=ones_t[0:1, :mw],
                             rhs=bout_t[0:1, n * NT:(n + 1) * NT],
                             start=False, stop=True)
            o_sb = work.tile([P, NT], F32, name="o_sb", tag="o_sb", bufs=3)
            nc.vector.tensor_copy(o_sb[:mw, :], ps_v[:mw, :])
            nc.gpsimd.dma_start(
                out_s[:, ta:tb, n * NT:(n + 1) * NT].transpose([1, 0, 2]),
                o_sb[:mw, :])

        # ---------------- pre-loop ----------------
        ctx_blk = [pool1.tile([P, KH, 16 * B], F16, name=f"cxb{i}")
                   for i in range(2)]
        stage_a(0)
        xg_q = {0: xg_prefetch(0), 1: xg_prefetch(1)}

        # ---------------- main loop ----------------
        c_prev = c0_sb
        for t in range(T):
            # gates: psum [P, 4B]; identity-matmul folds Xg in
            psg = psp.tile([P, CH * B], F32, name="psg", tag="psg", bufs=1)
            xg = xg_q.pop(t)
            nc.tensor.matmul(psg[:], lhsT=ident[:],
                             rhs=xg[:].rearrange("p c b -> p (c b)"),
                             start=True, stop=False, skip_group_check=True)
            for qq in range(CH):
                for k in range(KH):
                    rhs = (h0_t[:, k, :] if t == 0 else
                           hall4[:, k, t - 1, :])
                    nc.tensor.matmul(psg[:, qq * B:(qq + 1) * B],
                                     lhsT=whh[:, k, qq * P:(qq + 1) * P],
                                     rhs=rhs, start=False,
                                     stop=(qq == CH - 1 and k == KH - 1),
                                     skip_group_check=True)
            sfo = work.tile([P, 3 * B], F32, name="sfo", tag="sfo")
            nc.scalar.activation(sfo[:], psg[:, 0:3 * B], AF.Sigmoid)
            tg = work.tile([P, B], F32, name="tg", tag="tg")
            nc.scalar.activation(tg[:], psg[:, 3 * B:4 * B], AF.Tanh)
            t1 = work.tile([P, B], F32, name="t1", tag="t1")
            nc.vector.tensor_mul(t1[:], sfo[:, B:2 * B], c_prev[:])
            t2 = work.tile([P, B], F32, name="t2", tag="t2")
            nc.vector.tensor_mul(t2[:], sfo[:, 0:B], tg[:])
            c_new = state.tile([P, B], F32, name="c_new", tag="c_new")
            nc.vector.tensor_add(c_new[:], t1[:], t2[:])
            c_prev = c_new
            tc_t = work.tile([P, B], F32, name="tc_t", tag="tc_t")
            nc.scalar.activation(tc_t[:], c_new[:], AF.Tanh)
            h16 = work.tile([P, B], F16, name="h16", tag="h16")
            nc.vector.tensor_mul(h16[:], sfo[:, 2 * B:3 * B], tc_t[:])
            nc.sync.dma_start(cc_in[t][:, 0:B], h16[:])
            nc.gpsimd.collective_compute(
                "AllGather", ALU.bypass,
                replica_groups=[list(range(NCORES))],
                ins=[cc_in[t].opt()], outs=[cc_out[t].opt()])
            nc.sync.dma_start(
                hall4[:, :, t, :],
                cc_out[t][:, 0:B].rearrange("(k p) b -> p k b", p=P))
            if t in SHIP_SCHED:
                d0, nch = SHIP_SCHED[t]
                nc.sync.dma_start(
                    dectT[:, :, d0 * B:(d0 + nch) * B],
                    cc_out[t][:, B:(1 + nch) * B]
                    .rearrange("(k p) b -> p k b", p=P))

            # ---- filler ----
            if t + 2 < T:
                xg_q[t + 2] = xg_prefetch(t + 2)
            if t in STAGEA_STEPS:
                stage_a(STAGEA_STEPS[t])
            for (bi, j) in ATTN_SCHED.get(t, []):
                attn_scores(bi, j)
            for (bi, jp) in CTX_SCHED.get(t, []):
                attn_ctx(bi, jp)
            if t in DEC_SCHED:
                dec_blk(DEC_SCHED[t])
            if t + 1 in SHIP_SCHED:
                d0, nch = SHIP_SCHED[t + 1]
                nc.gpsimd.dma_start(cc_in[t + 1][:, B:(1 + nch) * B],
                                    dect_own[:, d0 * B:(d0 + nch) * B])
            for (g, n, ld) in VOCAB_SCHED.get(t, []):
                vocab_chunk(g, n, ld)

        # ---------------- tail ----------------
        # interleave post-block attention with vocab whose rows landed in-loop
        early = list(TAIL_VOCAB_EARLY)
        ei = 0
        for bi in POST_BLOCKS:
            for j in range(32):
                attn_scores(bi, j)
                if j % 2 == 1:
                    attn_ctx(bi, j // 2)
                if ei < len(early) and j % 2 == 1:
                    g, n = early[ei]
                    vocab_chunk(g, n, True)
                    ei += 1
            dec_blk(bi)
        for (g, n) in early[ei:]:
            vocab_chunk(g, n, True)
        assert TAIL_ROWS == list(range(TAIL_ROWS[0], TAIL_ROWS[-1] + 1))
        nc.sync.dma_start(
            fin_in[:], dect_own[:, TAIL_ROWS[0] * B:(TAIL_ROWS[-1] + 1) * B])
        nc.gpsimd.collective_compute(
            "AllGather", ALU.bypass, replica_groups=[list(range(NCORES))],
            ins=[fin_in.opt()], outs=[fin_out.opt()])
        nc.sync.dma_start(
            dectT[:, :, TAIL_ROWS[0] * B:(TAIL_ROWS[-1] + 1) * B],
            fin_out[:].rearrange("(k p) b -> p k b", p=P))
        tail_by_n = {}
        for (g, n) in TAIL_VOCAB_LATE:
            tail_by_n.setdefault(n, []).append(g)
        for n, gs in sorted(tail_by_n.items()):
            for i, g in enumerate(sorted(set(gs))):
                vocab_chunk(g, n, i == 0)
    nc.compile()
    return nc


_CACHE = {}


def _get_graph():
    if "nc" not in _CACHE:
        _CACHE["nc"] = build_graph()
    return _CACHE["nc"]


def _prep(tgt_input, hidden_state, cell_state, encoder_outputs,
          embedding, W_ih, W_hh, b_ih, b_hh, W_w, b_w, W_out, b_out):
    f32 = np.float32
    f16 = np.float16
    idx = np.asarray(tgt_input)[:, :-1].astype(np.int64)
    emb = np.asarray(embedding, f32)[idx]                    # [B, T, E]
    x_embT = np.ascontiguousarray(
        emb.transpose(2, 1, 0).reshape(E, R)).astype(f16)
    w_ihT = np.asarray(W_ih, f32).T                          # [E, G]
    w_hhT = np.asarray(W_hh, f32).T                          # [H, G]
    bias = (np.asarray(b_ih, f32) + np.asarray(b_hh, f32))
    h0T = np.ascontiguousarray(np.asarray(hidden_state, f32)[0].T).astype(f16)
    c0T = np.ascontiguousarray(np.asarray(cell_state, f32)[0].T)   # [H, B]
    enc = np.asarray(encoder_outputs, f32)                   # [B, S, H]
    enc_r = enc.astype(f16)
    encT_r = np.ascontiguousarray(
        enc.transpose(2, 1, 0)                               # [H, S, B]
        .reshape(KH, P, S, B).transpose(1, 3, 0, 2)          # [P, B, KH, S]
        .reshape(P, B * KH * S)).astype(f16)
    w_wT = np.ascontiguousarray(np.asarray(W_w, f32).T)      # [2H, H]
    b_w_a = np.asarray(b_w, f32)
    w_outT = np.asarray(W_out, f32).T                        # [H, V]
    b_out_a = np.asarray(b_out, f32)

    in_maps = []
    for m in range(NCORES):
        cols = np.concatenate([np.arange(Q_ORDER[q] * H + m * P,
                                         Q_ORDER[q] * H + m * P + P)
                               for q in range(4)])
        in_maps.append({
            "x_embT": x_embT,
            "wih_s": np.ascontiguousarray(w_ihT[:, cols]).astype(f16),
            "whh_s": np.ascontiguousarray(w_hhT[:, cols]).astype(f16),
            "bias_s": np.ascontiguousarray(bias[cols].reshape(CH, P).T),
            "h0T": h0T,
            "c0T_s": np.ascontiguousarray(c0T[m * P:(m + 1) * P, :]),
            "encT_r": encT_r,
            "enc_r": enc_r,
            "ww_s": np.ascontiguousarray(w_wT[:, m * P:(m + 1) * P]).astype(f16),
            "bw_s": np.ascontiguousarray(b_w_a[m * P:(m + 1) * P]).reshape(P, 1),
            "wout_s": np.ascontiguousarray(
                w_outT[:, m * VL:(m + 1) * VL]).astype(f16),
            "bout_s": np.ascontiguousarray(
                b_out_a[m * VL:(m + 1) * VL]).reshape(1, VL).astype(f16),
        })
    return in_maps


def kernel(**inputs) -> np.ndarray:
    nc = _get_graph()
    in_maps = _prep(**inputs)
    res = run_bass_kernel_spmd(nc, in_maps, list(range(NCORES)))
    outs = [res.results[m]["out_s"] for m in range(NCORES)]
    return np.concatenate(outs, axis=2)


# revision 16
# speedup vs baseline: 1.1201x; 1.0269x over previous
"""DecoderRNN Trainium2 kernel: 63-step LSTM + Luong attention + vocab projection.

Strategy (8 NeuronCores, SPMD), fp16 datapath (c-state/PSUM/softmax in f32):
  - Recurrence TP=8 over gate dims: each core owns 128 hidden dims x 4 gates
    (quarter order i,f,o,g so one sigmoid ACT covers i|f|o). Gates accumulate in
    ONE psum tile [P, 4B]; precomputed XgT enters via an identity-matmul.
    Per-step AllGather of the fp16 h-slice; payload [P, 3B] also piggybacks
    dect row-chunks (see below) so no extra collectives are needed.
  - Attention + W_w decoder: processed in t-blocks after the block's h has
    landed, spread across later steps as PE filler inside the AllGather gaps
    (also keeps the PE HAM-warm). W_w output is sharded by hidden chunk per
    core (per-core weight slice); the AllGather piggyback distributes dect so
    every core gets the full [H, R] dect for its vocab slice.
  - Vocab projection V-sharded (4000 cols/core), interleaved into the loop as
    dect rows land; out DMA per (t-group, n-tile) chunk.
  - Host side does layout-only prep; output is np.concatenate over V.
"""

import numpy as np
import ml_dtypes
from contextlib import ExitStack

import concourse.bass as bass
import concourse.bacc as bacc
import concourse.tile as tile
import concourse.mybir as mybir
from concourse import masks
from concourse.bass_utils import run_bass_kernel_spmd

F32 = mybir.dt.float32
F16 = mybir.dt.float16
AF = mybir.ActivationFunctionType
ALU = mybir.AluOpType

B, T, S = 32, 63, 64
V, E, H = 32000, 512, 1024
P = 128
NCORES = 8
R = T * B                      # 2016 rows, r = t*B + b
VL = V // NCORES               # 4000
KH = H // P                    # 8
KE = E // P                    # 4
CH = 4                         # owned gate chunks (i,f,o,g quarters)
NT = 500                       # vocab n-tile width
VN = VL // NT                  # 8
Q_ORDER = [0, 1, 3, 2]         # quarter -> pytorch gate index (i,f,o,g)

# attention blocks (start, end)
BLOCKS = [(0, 16), (16, 32), (32, 44), (44, 56), (56, 63)]
SPREADS = [4, 4, 4, 8, 8]
TGROUPS = [(4 * i, min(4 * i + 4, T)) for i in range(16)]

# ---------------- static schedule ----------------


def build_schedule():
    scores = {}    # step -> list of (blk_idx, b)
    ctxs = {}      # step -> list of (blk_idx, jpair)
    dec = {}       # step -> blk_idx
    ship = {}      # slot -> (d0, nchunks)
    land = {}      # row-chunk d -> landing step
    post_blocks = []
    for bi, (a, bnd) in enumerate(BLOCKS):
        start = bnd + 1
        spread = SPREADS[bi]
        steps_needed = (32 + spread - 1) // spread
        if start + steps_needed + 2 > T:
            post_blocks.append(bi)
            continue
        for j in range(32):
            st = start + j // spread
            scores.setdefault(st, []).append((bi, j))
            if j % 2 == 1:
                ctxs.setdefault(st + 1, []).append((bi, j // 2))
        dstep = start + steps_needed + 1
        dec[dstep] = bi
        d = a
        slot = dstep + 1
        while d < bnd:
            nch = min(2, bnd - d)
            if slot >= T:
                break
            ship[slot] = (d, nch)
            for dd in range(d, d + nch):
                land[dd] = slot + 1
            d += nch
            slot += 1
    tail_rows = [d for d in range(T) if d not in land]
    avail = {}
    for g, (ta, tb) in enumerate(TGROUPS):
        if all(d in land for d in range(ta, tb)):
            avail[g] = max(land[d] for d in range(ta, tb)) + 1
        else:
            avail[g] = None
    vocab = {}
    items = []
    ready_groups = sorted([g for g in avail if avail[g] is not None],
                          key=lambda g: avail[g])
    rounds = []
    cur = []
    for g in ready_groups:
        cur.append(g)
        if len(cur) == 2:
            rounds.append(cur)
            cur = []
    if cur:
        rounds.append(cur)
    for rnd in rounds:
        rstart = max(avail[g] for g in rnd)
        for n in range(VN):
            for i, g in enumerate(rnd):
                items.append((rstart, g, n, i == 0))
    items.sort(key=lambda x: x[0])
    qi = 0
    for t in range(T):
        quota = 2 if t < 40 else 3
        cnt = 0
        while qi < len(items) and cnt < quota and items[qi][0] <= t:
            _, g, n, ld = items[qi]
            vocab.setdefault(t, []).append((g, n, ld))
            qi += 1
            cnt += 1
    tail_vocab_early = []   # runnable before fin AG (rows landed in-loop)
    tail_vocab_late = []
    for (_, g, n, ld) in items[qi:]:
        tail_vocab_early.append((g, n))
    for g in [g for g in avail if avail[g] is None]:
        for n in range(VN):
            tail_vocab_late.append((g, n))
    return (scores, ctxs, dec, ship, tail_rows, vocab,
            tail_vocab_early, tail_vocab_late, post_blocks)


(ATTN_SCHED, CTX_SCHED, DEC_SCHED, SHIP_SCHED, TAIL_ROWS, VOCAB_SCHED,
 TAIL_VOCAB_EARLY, TAIL_VOCAB_LATE, POST_BLOCKS) = build_schedule()
POST_BLOCKS = sorted(set(POST_BLOCKS))
STAGEA_STEPS = {2: 1, 8: 2, 12: 3}   # step -> stage-A window (window 0 pre-loop)
AW = [(0, 512), (512, 1024), (1024, 1536), (1536, 2016)]


def build_graph():
    nc = bacc.Bacc("TRN2", target_bir_lowering=False, debug=False,
                   num_devices=NCORES)

    def inp(name, shape, dtype):
        return nc.dram_tensor(name, list(shape), dtype, kind="ExternalInput").ap()

    x_embT = inp("x_embT", [E, R], F16)
    wih_s = inp("wih_s", [E, CH * P], F16)
    whh_s = inp("whh_s", [H, CH * P], F16)
    bias_s = inp("bias_s", [P, CH], F32)
    h0T = inp("h0T", [H, B], F16)
    c0T_s = inp("c0T_s", [P, B], F32)
    encT_r = inp("encT_r", [P, B * KH * S], F16)   # [p, b, k, s]
    enc_r = inp("enc_r", [B, S, H], F16)
    ww_s = inp("ww_s", [2 * H, P], F16)            # W_w.T cols for own mo chunk
    bw_s = inp("bw_s", [P, 1], F32)
    wout_s = inp("wout_s", [H, VL], F16)
    bout_s = inp("bout_s", [1, VL], F16)
    out_s = nc.dram_tensor("out_s", [B, T, VL], F32, kind="ExternalOutput").ap()

    with tile.TileContext(nc) as tc, ExitStack() as ctx:
        pool1 = ctx.enter_context(tc.tile_pool(name="pool1", bufs=1))
        stream = ctx.enter_context(tc.tile_pool(name="stream", bufs=3))
        work = ctx.enter_context(tc.tile_pool(name="work", bufs=2))
        state = ctx.enter_context(tc.tile_pool(name="state", bufs=2))
        psp = ctx.enter_context(tc.tile_pool(name="psp", bufs=1, space="PSUM"))
        dram = ctx.enter_context(tc.tile_pool(name="dram", bufs=1, space="DRAM"))

        # ---------------- resident tiles ----------------
        hall = pool1.tile([P, KH, R], F16, name="hall")
        hall4 = hall.rearrange("p k (t b) -> p k t b", b=B)
        dectT = pool1.tile([P, KH, R], F16, name="dectT")
        dect_own = pool1.tile([P, R], F16, name="dect_own")
        whh = pool1.tile([P, KH, CH * P], F16, name="whh")
        nc.sync.dma_start(whh[:], whh_s.rearrange("(k p) c -> p k c", p=P))
        wih = pool1.tile([P, KE, CH * P], F16, name="wih")
        nc.sync.dma_start(wih[:], wih_s.rearrange("(k p) c -> p k c", p=P))
        bias_t = pool1.tile([P, CH], F32, name="bias_t")
        nc.sync.dma_start(bias_t[:], bias_s[:])
        encT_sb = pool1.tile([P, B, KH, S], F16, name="encT_sb")
        nc.sync.dma_start(encT_sb[:],
                          encT_r.rearrange("p (b k s) -> p b k s", b=B, k=KH))
        ww_sb = pool1.tile([P, 2 * KH, P], F16, name="ww_sb")
        nc.sync.dma_start(ww_sb[:], ww_s.rearrange("(j p) m -> p j m", p=P))
        bw_t = pool1.tile([P, 1], F32, name="bw_t")
        nc.sync.dma_start(bw_t[:], bw_s[:])
        bout_t = pool1.tile([1, VL], F16, name="bout_t")
        nc.sync.dma_start(bout_t[:], bout_s[:])
        ones_t = pool1.tile([1, P], F16, name="ones_t")
        nc.gpsimd.memset(ones_t[:], 1.0)
        h0_t = pool1.tile([P, KH, B], F16, name="h0_t")
        nc.sync.dma_start(h0_t[:], h0T.rearrange("(k p) b -> p k b", p=P))
        ident = pool1.tile([P, P], F16, name="ident")
        masks.make_identity(nc, ident[:])
        c0_sb = pool1.tile([P, B], F32, name="c0_sb")
        nc.sync.dma_start(c0_sb[:], c0T_s[:])

        xg_dram = dram.tile([CH, P, R], F16, name="xg_dram")
        cc_in = [dram.tile([P, 3 * B], F16, name=f"cc_in{i}") for i in range(T)]
        cc_out = [dram.tile([NCORES * P, 3 * B], F16, name=f"cc_out{i}",
                            addr_space="Shared") for i in range(T)]
        NTAIL = len(TAIL_ROWS)
        fin_in = dram.tile([P, NTAIL * B], F16, name="fin_in")
        fin_out = dram.tile([NCORES * P, NTAIL * B], F16, name="fin_out",
                            addr_space="Shared")

        # ---------------- helpers ----------------
        def stage_a(w):
            a, bnd = AW[w]
            nw = bnd - a
            xt = stream.tile([P, KE, 512], F16, name="xa", tag="xa", bufs=2)
            nc.gpsimd.dma_start(xt[:, :, :nw],
                                x_embT.rearrange("(k p) r -> p k r", p=P)[:, :, a:bnd])
            for c in range(CH):
                ps = psp.tile([P, 512], F32, name="ps_a", tag="mm")
                for k in range(KE):
                    nc.tensor.matmul(ps[:, :nw], lhsT=wih[:, k, c * P:(c + 1) * P],
                                     rhs=xt[:, k, :nw],
                                     start=(k == 0), stop=(k == KE - 1))
                xga = work.tile([P, 512], F16, name="xga", tag="xga", bufs=2)
                nc.scalar.activation(xga[:, :nw], ps[:, :nw], AF.Identity,
                                     bias=bias_t[:, c:c + 1])
                nc.gpsimd.dma_start(xg_dram[c, :, a:bnd], xga[:, :nw])

        def xg_prefetch(t):
            xg = stream.tile([P, CH, B], F16, name="xg", tag="xg", bufs=4)
            nc.gpsimd.dma_start(
                xg[:], xg_dram[:, :, t * B:(t + 1) * B].rearrange("c p b -> p c b"))
            return xg

        ec_tiles = {}
        pn2_tiles = {}

        def attn_scores(bi, j):
            blk_a, blk_b = BLOCKS[bi]
            w = blk_b - blk_a
            if j % 2 == 0 and (bi, j // 2) not in ec_tiles:
                b0 = j
                ec = stream.tile([2 * S, H], F16, name="ec", tag="ec", bufs=4)
                nc.gpsimd.dma_start(ec[0:S, :], enc_r[b0, :, :])
                nc.gpsimd.dma_start(ec[S:2 * S, :], enc_r[b0 + 1, :, :])
                ec_tiles[(bi, j // 2)] = ec
            ps_sc = psp.tile([P, S], F32, name="ps_sc", tag="sc", bufs=3)
            for k in range(KH):
                nc.tensor.matmul(ps_sc[:w, :],
                                 lhsT=hall4[:, k, blk_a:blk_b, j],
                                 rhs=encT_sb[:, j, k, :],
                                 start=(k == 0), stop=(k == KH - 1))
            probs = work.tile([P, S], F32, name="probs", tag="probs")
            ssum = work.tile([P, 1], F32, name="ssum", tag="ssum")
            nc.scalar.activation(probs[:w, :], ps_sc[:w, :], AF.Exp,
                                 accum_out=ssum[:w])
            lgd = work.tile([P, 1], F32, name="lgd", tag="lgd")
            nc.scalar.activation(lgd[:w], ssum[:w], AF.Ln)
            rec = work.tile([P, 1], F32, name="rec", tag="rec")
            nc.scalar.activation(rec[:w], lgd[:w], AF.Exp, scale=-1.0)
            pn2 = pn2_tiles.get((bi, j // 2))
            if pn2 is None:
                pn2 = work.tile([P, 2, S], F16, name="pn2", tag="pn2", bufs=6)
                pn2_tiles[(bi, j // 2)] = pn2
            nc.scalar.mul(pn2[:w, j % 2, :], probs[:w, :], rec[:w])

        def attn_ctx(bi, jp):
            blk_a, blk_b = BLOCKS[bi]
            w = blk_b - blk_a
            ec = ec_tiles.pop((bi, jp))
            pn2 = pn2_tiles.pop((bi, jp))
            ps_at = psp.tile([P, 16], F16, name="ps_at", tag="atx")
            nc.tensor.transpose(
                ps_at[:, :w],
                pn2.rearrange("p a s -> p (a s)")[:w, :],
                ident[:w, :w])
            attnT = work.tile([P, 16], F16, name="attnT", tag="attnT", bufs=2)
            nc.vector.tensor_copy(attnT[:, :w], ps_at[:, :w])
            for jj in range(2):
                bb = jp * 2 + jj
                ps_cx = psp.tile([P, KH, 16], F32, name="ps_cx", tag="atx")
                for k in range(KH):
                    nc.tensor.matmul(ps_cx[:, k, :w],
                                     lhsT=ec[jj * S:(jj + 1) * S,
                                             k * P:(k + 1) * P],
                                     rhs=attnT[jj * S:(jj + 1) * S, :w],
                                     start=True, stop=True)
                cxb = ctx_blk[bi % 2]
                cxr = cxb.rearrange("p k (t b) -> p k t b", b=B)
                nc.vector.tensor_copy(cxr[:, :, :w, bb], ps_cx[:, :, :w])

        def dec_blk(bi):
            blk_a, blk_b = BLOCKS[bi]
            w = blk_b - blk_a
            cxb = ctx_blk[bi % 2]
            ps_d = psp.tile([P, 512], F32, name="ps_d", tag="mm")
            for j in range(2 * KH):
                rhs = (hall[:, j, blk_a * B:blk_b * B] if j < KH
                       else cxb[:, j - KH, :w * B])
                nc.tensor.matmul(ps_d[:, :w * B], lhsT=ww_sb[:, j, :], rhs=rhs,
                                 start=(j == 0), stop=(j == 2 * KH - 1))
            nc.scalar.activation(dect_own[:, blk_a * B:blk_b * B], ps_d[:, :w * B],
                                 AF.Tanh, bias=bw_t[:, 0:1])

        wo_tiles = {}

        def vocab_chunk(g, n, load):
            ta, tb = TGROUPS[g]
            mw = (tb - ta) * B
            wo = wo_tiles.get(n % 4) if not load else None
            if load or wo is None:
                wo = stream.tile([P, KH, NT], F16, name="wo", tag=f"wo{n % 4}",
                                 bufs=1)
                nc.gpsimd.dma_start(
                    wo[:], wout_s[:, n * NT:(n + 1) * NT]
                    .rearrange("(k p) v -> p k v", p=P))
                wo_tiles[n % 4] = wo
            ps_v = psp.tile([P, NT], F32, name="ps_v", tag="pv", bufs=2)
            for k in range(KH):
                nc.tensor.matmul(ps_v[:mw, :], lhsT=dectT[:, k, ta * B:tb * B],
                                 rhs=wo[:, k, :], start=(k == 0), stop=False)
            nc.tensor.matmul(ps_v[:mw, :], lhsT=ones_t[0:1, :mw],
                             rhs=bout_t[0:1, n * NT:(n + 1) * NT],
                             start=False, stop=True)
            o_sb = work.tile([P, NT], F32, name="o_sb", tag="o_sb", bufs=3)
            nc.vector.tensor_copy(o_sb[:mw, :], ps_v[:mw, :])
            nc.gpsimd.dma_start(
                out_s[:, ta:tb, n * NT:(n + 1) * NT].transpose([1, 0, 2]),
                o_sb[:mw, :])

        # ---------------- pre-loop ----------------
        ctx_blk = [pool1.tile([P, KH, 16 * B], F16, name=f"cxb{i}")
                   for i in range(2)]
        stage_a(0)
        xg_q = {0: xg_prefetch(0), 1: xg_prefetch(1)}

        # ---------------- main loop ----------------
        c_prev = c0_sb
        for t in range(T):
            # gates: psum [P, 4B]; identity-matmul folds Xg in
            psg = psp.tile([P, CH * B], F32, name="psg", tag="psg", bufs=1)
            xg = xg_q.pop(t)
            nc.tensor.matmul(psg[:], lhsT=ident[:],
                             rhs=xg[:].rearrange("p c b -> p (c b)"),
                             start=True, stop=False, skip_group_check=True)
            for qq in range(CH):
                for k in range(KH):
                    rhs = (h0_t[:, k, :] if t == 0 else
                           hall4[:, k, t - 1, :])
                    nc.tensor.matmul(psg[:, qq * B:(qq + 1) * B],
                                     lhsT=whh[:, k, qq * P:(qq + 1) * P],
                                     rhs=rhs, start=False,
                                     stop=(qq == CH - 1 and k == KH - 1),
                                     skip_group_check=True)
            sfo = work.tile([P, 3 * B], F32, name="sfo", tag="sfo")
            nc.scalar.activation(sfo[:], psg[:, 0:3 * B], AF.Sigmoid)
            tg = work.tile([P, B], F32, name="tg", tag="tg")
            nc.scalar.activation(tg[:], psg[:, 3 * B:4 * B], AF.Tanh)
            t1 = work.tile([P, B], F32, name="t1", tag="t1")
            nc.vector.tensor_mul(t1[:], sfo[:, B:2 * B], c_prev[:])
            t2 = work.tile([P, B], F32, name="t2", tag="t2")
            nc.vector.tensor_mul(t2[:], sfo[:, 0:B], tg[:])
            c_new = state.tile([P, B], F32, name="c_new", tag="c_new")
            nc.vector.tensor_add(c_new[:], t1[:], t2[:])
            c_prev = c_new
            tc_t = work.tile([P, B], F32, name="tc_t", tag="tc_t")
            nc.scalar.activation(tc_t[:], c_new[:], AF.Tanh)
            h16 = work.tile([P, B], F16, name="h16", tag="h16")
            nc.vector.tensor_mul(h16[:], sfo[:, 2 * B:3 * B], tc_t[:])
            nc.sync.dma_start(cc_in[t][:, 0:B], h16[:])
            nc.gpsimd.collective_compute(
                "AllGather", ALU.bypass,
                replica_groups=[list(range(NCORES))],
                ins=[cc_in[t].opt()], outs=[cc_out[t].opt()])
            nc.sync.dma_start(
                hall4[:, :, t, :],
                cc_out[t][:, 0:B].rearrange("(k p) b -> p k b", p=P))
            if t in SHIP_SCHED:
                d0, nch = SHIP_SCHED[t]
                nc.sync.dma_start(
                    dectT[:, :, d0 * B:(d0 + nch) * B],
                    cc_out[t][:, B:(1 + nch) * B]
                    .rearrange("(k p) b -> p k b", p=P))

            # ---- filler ----
            if t + 2 < T:
                xg_q[t + 2] = xg_prefetch(t + 2)
            if t in STAGEA_STEPS:
                stage_a(STAGEA_STEPS[t])
            for (bi, j) in ATTN_SCHED.get(t, []):
                attn_scores(bi, j)
            for (bi, jp) in CTX_SCHED.get(t, []):
                attn_ctx(bi, jp)
            if t in DEC_SCHED:
                dec_blk(DEC_SCHED[t])
            if t + 1 in SHIP_SCHED:
                d0, nch = SHIP_SCHED[t + 1]
                nc.gpsimd.dma_start(cc_in[t + 1][:, B:(1 + nch) * B],
                                    dect_own[:, d0 * B:(d0 + nch) * B])
            for (g, n, ld) in VOCAB_SCHED.get(t, []):
                vocab_chunk(g, n, ld)

        # ---------------- tail ----------------
        # interleave post-block attention with vocab whose rows landed in-loop
        early = list(TAIL_VOCAB_EARLY)
        ei = 0
        for bi in POST_BLOCKS:
            for j in range(32):
                attn_scores(bi, j)
                if j % 2 == 1:
                    attn_ctx(bi, j // 2)
                if ei < len(early) and j % 2 == 1:
                    g, n = early[ei]
                    vocab_chunk(g, n, True)
                    ei += 1
            dec_blk(bi)
        for (g, n) in early[ei:]:
            vocab_chunk(g, n, True)
        assert TAIL_ROWS == list(range(TAIL_ROWS[0], TAIL_ROWS[-1] + 1))
        nc.sync.dma_start(
            fin_in[:], dect_own[:, TAIL_ROWS[0] * B:(TAIL_ROWS[-1] + 1) * B])
        nc.gpsimd.collective_compute(
            "AllGather", ALU.bypass, replica_groups=[list(range(NCORES))],
            ins=[fin_in.opt()], outs=[fin_out.opt()])
        nc.sync.dma_start(
            dectT[:, :, TAIL_ROWS[0] * B:(TAIL_ROWS[-1] + 1) * B],
            fin_out[:].rearrange("(k p) b -> p k b", p=P))
        tail_by_n = {}
        for (g, n) in TAIL_VOCAB_LATE:
            tail_by_n.setdefault(n, []).append(g)
        for n, gs in sorted(tail_by_n.items()):
            for i, g in enumerate(sorted(set(gs))):
                vocab_chunk(g, n, i == 0)
    nc.compile()
    return nc


_CACHE = {}


def _get_graph():
    if "nc" not in _CACHE:
        _CACHE["nc"] = build_graph()
    return _CACHE["nc"]


def _prep(tgt_input, hidden_state, cell_state, encoder_outputs,
          embedding, W_ih, W_hh, b_ih, b_hh, W_w, b_w, W_out, b_out):
    f32 = np.float32
    f16 = np.float16
    idx = np.asarray(tgt_input)[:, :-1].astype(np.int64)
    emb = np.asarray(embedding, f32)[idx]                    # [B, T, E]
    x_embT = np.ascontiguousarray(
        emb.transpose(2, 1, 0).reshape(E, R)).astype(f16)
    w_ihT = np.asarray(W_ih, f32).T                          # [E, G]
    w_hhT = np.asarray(W_hh, f32).T                          # [H, G]
    bias = (np.asarray(b_ih, f32) + np.asarray(b_hh, f32))
    h0T = np.ascontiguousarray(np.asarray(hidden_state, f32)[0].T).astype(f16)
    c0T = np.ascontiguousarray(np.asarray(cell_state, f32)[0].T)   # [H, B]
    enc = np.asarray(encoder_outputs, f32)                   # [B, S, H]
    enc_r = enc.astype(f16)
    encT_r = np.ascontiguousarray(
        enc.transpose(2, 1, 0)                               # [H, S, B]
        .reshape(KH, P, S, B).transpose(1, 3, 0, 2)          # [P, B, KH, S]
        .reshape(P, B * KH * S)).astype(f16)
    w_wT = np.ascontiguousarray(np.asarray(W_w, f32).T)      # [2H, H]
    b_w_a = np.asarray(b_w, f32)
    w_outT = np.asarray(W_out, f32).T                        # [H, V]
    b_out_a = np.asarray(b_out, f32)

    in_maps = []
    for m in range(NCORES):
        cols = np.concatenate([np.arange(Q_ORDER[q] * H + m * P,
                                         Q_ORDER[q] * H + m * P + P)
                               for q in range(4)])
        in_maps.append({
            "x_embT": x_embT,
            "wih_s": np.ascontiguousarray(w_ihT[:, cols]).astype(f16),
            "whh_s": np.ascontiguousarray(w_hhT[:, cols]).astype(f16),
            "bias_s": np.ascontiguousarray(bias[cols].reshape(CH, P).T),
            "h0T": h0T,
            "c0T_s": np.ascontiguousarray(c0T[m * P:(m + 1) * P, :]),
            "encT_r": encT_r,
            "enc_r": enc_r,
            "ww_s": np.ascontiguousarray(w_wT[:, m * P:(m + 1) * P]).astype(f16),
            "bw_s": np.ascontiguousarray(b_w_a[m * P:(m + 1) * P]).reshape(P, 1),
            "wout_s": np.ascontiguousarray(
                w_outT[:, m * VL:(m + 1) * VL]).astype(f16),
            "bout_s": np.ascontiguousarray(
                b_out_a[m * VL:(m + 1) * VL]).reshape(1, VL).astype(f16),
        })
    return in_maps


def kernel(**inputs) -> np.ndarray:
    nc = _get_graph()
    in_maps = _prep(**inputs)
    res = run_bass_kernel_spmd(nc, in_maps, list(range(NCORES)))
    outs = [res.results[m]["out_s"] for m in range(NCORES)]
    return np.concatenate(outs, axis=2)
